# revision 22
# baseline (speedup 1.0000x reference)
"""Weighted two-sided chamfer loss (AutoDecLoss) for Trainium2 -- 8 cores.

Strategy
--------
Data-parallel over the batch: core b computes the full [N=2048, M=4096]
chamfer block of batch element b; the host averages the 8 per-core scalars.

Distances come straight off the PE via augmented features,

    d[n, m] = sum_k X[k, n] * Y[k, m],
    X = [x^2, -2x, 1] rows, Y = [1, y, y^2] rows (K = 9),

computed as a single K=27 bf16 matmul using a compensated hi/lo split
(A ~ Ah+Al, B ~ Bh+Bl; A.T B ~ Ah.T Bh + Ah.T Bl + Al.T Bh with stacked
operands [Ah;Ah;Al] x [Bh;Bl;Bh]) -- 1 cycle/row instead of fp32's 4, at
~1e-5 end-to-end loss error.

Row mins (forward) use a custom DVE micro-op (registered at import time):
out = min(in0, in1), accum_out = min(seed, min(out)) -- two input streams
per cycle (one PSUM + one SBUF copied by the otherwise-idle ACT engine),
double the throughput of tensor_reduce.  The backward direction runs a
second matmul pass in [m, n] layout with X pre-scaled by 1/max(w, 1e-3).

Setup (feature build, bf16 splits, weight reciprocal via a PE transpose
round-trip) is pipelined: the x/y-half0 chains lead, y-half1 and the
scaled-X chain run on GPSIMD underneath the forward-g0 compute.
"""

import re

import numpy as np

import concourse.bacc as bacc
import concourse.mybir as mybir
import concourse.tile as tile
from concourse import dve_ops, masks
from concourse.bass_utils import run_bass_kernel_spmd
from concourse.dve_spec import C0, Spec, Src0, Src1, minn
from concourse.dve_table_gen import dve_ver_for


_OP_NAME = "MIN_MIN_REDUCE_ANT"


def _ref(in0, in1, s0, s1, imm2):
    out = np.minimum(in0.astype(np.float32), in1.astype(np.float32))
    P = out.shape[0]
    body = out.reshape(P, -1)
    seed = np.asarray(s0, np.float32).reshape(-1, 1)
    acc = np.minimum(np.minimum.reduce(body, axis=-1, keepdims=True), seed)
    return out, acc


def get_min_min_reduce():
    for op in dve_ops.OPS:
        if op.name == _OP_NAME:
            return op
    spec = Spec(body=minn(Src0, Src1), accum=minn, accum_init=C0, reference=_ref)
    ver = dve_ver_for("TRN2")
    probe = dve_ops.DveOp(_OP_NAME, spec, subdim=False, uops_sha={})
    # register a row before compiling (compile needs the sub-opcode)
    row = dve_ops._CUSTOM_DVE_ROW_BASE + len(dve_ops.OPS)
    dve_ops._SUB_OPCODE_FOR_NAME[_OP_NAME] = row
    shas = {}
    for v in ("v3", "v4"):
        try:
            probe.compile(v)
            shas[v] = probe.uops_sha.get(v)
        except ValueError as e:
            m = re.search(rf"{v}: ([0-9a-f]+)", str(e))
            if not m:
                raise
            shas[v] = m.group(1)
    op = dve_ops.DveOp(_OP_NAME, spec, subdim=False, uops_sha=shas)
    dve_ops.OPS.append(op)
    dve_ops.CUSTOM_DVE_SPECS[_OP_NAME] = spec
    assert dve_ops.get_dve_sub_opcode(_OP_NAME) == row
    assert row < 0x20
    assert ver in shas
    return op


def min_min_reduce(nc, out, in0, in1, init, accum_out):
    op = get_min_min_reduce()
    return nc.vector._custom_dve(op, out=out, in0=in0, in1=in1, s0=init,
                                 accum_out=accum_out)


B, N, M = 8, 2048, 4096
NT = N // 128
MT = M // 128
CHAMFER_EPS = 1e-6
MIN_BW = 1e-3
BIG = 3.0e38

F32 = mybir.dt.float32
BF16 = mybir.dt.bfloat16
MIN = mybir.AluOpType.min
ADD = mybir.AluOpType.add
MULT = mybir.AluOpType.mult
SUB = mybir.AluOpType.subtract
AX = mybir.AxisListType.X
COPY = mybir.ActivationFunctionType.Copy


def build_nc():
    nc = bacc.Bacc("TRN2", target_bir_lowering=False, debug=False, num_devices=8)
    xT = nc.dram_tensor("xT", [3, N], F32, kind="ExternalInput")
    yT = nc.dram_tensor("yT", [3, M], F32, kind="ExternalInput")
    wT = nc.dram_tensor("wT", [128, NT], F32, kind="ExternalInput")
    sc = nc.dram_tensor("sc", [6, 1], F32, kind="ExternalInput")
    out = nc.dram_tensor("loss", [1, 1], F32, kind="ExternalOutput")

    HM = M // 2

    with tile.TileContext(nc) as tc:
        with (
            tc.tile_pool(name="feat", bufs=1) as fpool,
            tc.tile_pool(name="small", bufs=1) as spool,
        ):
            # ---------------- t=0: constants, no-dep work ----------------
            # block rows: X = [x^2(0-2), -2x(3-5), 1(6-8)]
            #             Y = [1(0-2),    y(3-5), y^2(6-8)]
            # X27 = (h,h,l); Y27 = (h,l,h); XS27 = (h,h,l) of scaled X.
            X27 = fpool.tile([27, N], BF16, tag="X27")
            Y27 = fpool.tile([27, M], BF16, tag="Y27")
            XS27 = fpool.tile([27, N], BF16, tag="XS27")
            nc.gpsimd.memset(X27[:], 1.0)
            nc.gpsimd.memset(Y27[:], 1.0)
            zer3 = fpool.tile([3, M], BF16, tag="zer3")
            nc.vector.memset(zer3[:].bitcast(F32), 0.0)

            sc6 = spool.tile([6, 1], F32, tag="sc6")
            nc.sync.dma_start(sc6[:], sc[:])
            wN = spool.tile([128, NT], F32, tag="wN")
            nc.sync.dma_start(wN[:], wT[:])
            ident = spool.tile([128, 128], BF16, tag="ident")
            masks.make_identity(nc, ident[:])
            identf = spool.tile([128, 128], F32, tag="identf")
            masks.make_identity(nc, identf[:])

            # -------- w / r wide-layout ops (lead, DVE) --------
            wc = spool.tile([128, NT], F32, tag="wc")
            nc.vector.tensor_scalar_max(wc[:], wN[:], MIN_BW)
            rw = spool.tile([128, NT], F32, tag="rw")
            nc.vector.reciprocal(rw[:], wc[:])
            rwh = spool.tile([128, NT], BF16, tag="rwh")
            nc.vector.tensor_copy(rwh[:], rw[:])
            rwl = spool.tile([128, NT], BF16, tag="rwl")
            nc.vector.tensor_tensor(rwl[:], rw[:], rwh[:], op=SUB)
            ones6 = spool.tile([1, 6], F32, tag="ones6")
            nc.vector.memset(ones6[:], 1.0)

            # ---------------- y-side tiles/chain ----------------
            yr = fpool.tile([3, M], F32, tag="yr")
            YT6 = fpool.tile([6, M], F32, tag="YT6")   # [y^2, y]
            yhi6 = fpool.tile([6, M], BF16, tag="yhi6")
            ylo6 = fpool.tile([6, M], BF16, tag="ylo6")

            def y_chain(h, sq_eng, hi_eng, lo_tt, q0, q1, q2):
                cs = slice(h * HM, (h + 1) * HM)
                q0.dma_start(yr[:, cs], yT[:, cs])
                q1.dma_start(YT6[3:6, cs], yT[:, cs])
                sq_eng(YT6[0:3, cs], yr[:, cs])
                hi_eng(yhi6[:, cs], YT6[:, cs])
                lo_tt(ylo6[:, cs], YT6[:, cs], yhi6[:, cs], op=SUB)
                q0.dma_start(Y27[3:6, cs], yhi6[3:6, cs])
                q1.dma_start(Y27[6:9, cs], yhi6[0:3, cs])
                q2.dma_start(Y27[12:15, cs], ylo6[3:6, cs])
                q0.dma_start(Y27[15:18, cs], ylo6[0:3, cs])
                q1.dma_start(Y27[21:24, cs], yhi6[3:6, cs])
                q2.dma_start(Y27[24:27, cs], yhi6[0:3, cs])

            y_chain(0, nc.scalar.square,
                    nc.vector.tensor_copy,
                    nc.vector.tensor_tensor,
                    nc.sync, nc.gpsimd, nc.sync)

            # ---------------- x-side chain (lead, ACT+DVE) ----------------
            xr = fpool.tile([3, N], F32, tag="xr")
            nc.sync.dma_start(xr[:], xT[:])
            XT6 = fpool.tile([6, N], F32, tag="XT6")   # [x^2, x]
            nc.sync.dma_start(XT6[3:6, :], xT[:])
            nc.scalar.square(XT6[0:3, :], xr[:])
            xhi6 = fpool.tile([6, N], BF16, tag="xhi6")
            nc.vector.tensor_scalar_mul(xhi6[:], XT6[:], sc6[:])
            xlo6 = fpool.tile([6, N], BF16, tag="xlo6")
            nc.vector.scalar_tensor_tensor(xlo6[:], XT6[:], sc6[:], xhi6[:],
                                           op0=MULT, op1=SUB)
            nc.sync.dma_start(X27[0:6, :], xhi6[:])
            nc.gpsimd.dma_start(X27[9:15, :], xhi6[:])
            nc.sync.dma_start(X27[18:24, :], xlo6[:])
            nc.gpsimd.dma_start(X27[24:27, :], zer3[:, 0:N])
            nc.gpsimd.dma_start(Y27[9:12, :], zer3[:])




            # ---------------- accumulators ----------------
            minf2 = spool.tile([128, 2 * NT], F32, tag="minf2")
            minb = spool.tile([128, MT], F32, tag="minb")
            fin = spool.tile([128, 3], F32, tag="fin")
            onescol = spool.tile([128, 1], F32, tag="onescol")
            nc.vector.memset(onescol[:], 1.0)

            def mm27(ps, lhsT, rhs_full, f0, fw):
                for k in range(fw // 512):
                    nc.tensor.matmul(ps[:, k * 512:(k + 1) * 512], lhsT,
                                     rhs_full[:, f0 + k * 512:f0 + (k + 1) * 512],
                                     start=True, stop=True)

            # ---------------- main loops ----------------
            with (
                tc.tile_pool(name="psum_main", bufs=4, space="PSUM") as mpool,
                tc.tile_pool(name="scratch", bufs=8) as scpool,
            ):
                # r-chain PSUM work uses main-pool slots (no pool barrier)
                ps_r = mpool.tile([NT, 128], F32, tag="d")
                nc.tensor.transpose(ps_r[:], rw[:], identf[:])
                sb_r = spool.tile([NT, 128], F32, tag="sb_r")
                nc.vector.tensor_copy(sb_r[:], ps_r[:])
                r_row = spool.tile([1, N], F32, tag="r_row")
                nc.gpsimd.dma_start(r_row[:], sb_r[:])

                ps_rh = mpool.tile([NT, 128], BF16, tag="d")
                nc.tensor.transpose(ps_rh[:], rwh[:], ident[:])
                ps_rl = mpool.tile([NT, 128], BF16, tag="d")
                nc.tensor.transpose(ps_rl[:], rwl[:], ident[:])
                sb_rh = spool.tile([NT, 128], BF16, tag="sb_rh")
                sb_rl = spool.tile([NT, 128], BF16, tag="sb_rl")
                nc.vector.tensor_copy(sb_rh[:], ps_rh[:])
                nc.vector.tensor_copy(sb_rl[:], ps_rl[:])

                XS6s = fpool.tile([6, N], F32, tag="XS6s")
                for hh in range(2):
                    R6h = mpool.tile([6, 1024], F32, tag="d")
                    for k in range(2):
                        f0 = hh * 1024 + k * 512
                        nc.tensor.matmul(R6h[:, k * 512:(k + 1) * 512],
                                         ones6[:], r_row[:, f0:f0 + 512],
                                         start=True, stop=True)
                    nc.vector.scalar_tensor_tensor(
                        XS6s[:, hh * 1024:(hh + 1) * 1024],
                        XT6[:, hh * 1024:(hh + 1) * 1024], sc6[:], R6h[:],
                        op0=MULT, op1=MULT)

                def reduce_block(lhsT, rhs, f0, acc_col):
                    psQ = mpool.tile([128, 1024], F32, tag="d")
                    psP = mpool.tile([128, 1024], F32, tag="d")
                    mm27(psQ, lhsT, rhs, f0 + 1024, 1024)
                    sbQ = scpool.tile([128, 1024], F32, tag="sbq")
                    nc.scalar.copy(sbQ[:], psQ[:])
                    mm27(psP, lhsT, rhs, f0, 1024)
                    tout = scpool.tile([128, 1024], F32, tag="tout")
                    min_min_reduce(nc, tout[:], psP[:], sbQ[:], BIG, acc_col)

                # ---- forward g=0 (needs only Y27 half 0) ----
                for c in range(NT):
                    reduce_block(X27[:, c * 128:(c + 1) * 128], Y27,
                                 0, minf2[:, c:c + 1])

                # ---- overlapped under fwd-g0: y-half1 (gpsimd) ----
                y_chain(1,
                        lambda o, i: nc.gpsimd.tensor_tensor(o, i, i, op=MULT),
                        nc.gpsimd.tensor_copy,
                        nc.gpsimd.tensor_tensor,
                        nc.scalar, nc.gpsimd, nc.sync)

                # ---- overlapped: XS bf16 split (gpsimd) + r rows ----
                xsh6 = fpool.tile([6, N], BF16, tag="xsh6")
                nc.gpsimd.tensor_copy(xsh6[:], XS6s[:])
                xsl6 = fpool.tile([6, N], BF16, tag="xsl6")
                nc.gpsimd.tensor_tensor(xsl6[:], XS6s[:], xsh6[:], op=SUB)
                nc.sync.dma_start(XS27[0:6, :], xsh6[:])
                nc.gpsimd.dma_start(XS27[9:15, :], xsh6[:])
                nc.sync.dma_start(XS27[18:24, :], xsl6[:])
                for j in range(3):
                    nc.gpsimd.dma_start(XS27[6 + j:7 + j, :], sb_rh[:])
                    nc.sync.dma_start(XS27[15 + j:16 + j, :], sb_rh[:])
                    nc.sync.dma_start(XS27[24 + j:25 + j, :], sb_rl[:])

                # ---- forward g=1 ----
                for c in range(NT):
                    reduce_block(X27[:, c * 128:(c + 1) * 128], Y27,
                                 2048, minf2[:, NT + c:NT + c + 1])

                # ---- backward ----
                for c in range(MT):
                    reduce_block(Y27[:, c * 128:(c + 1) * 128], XS27,
                                 0, minb[:, c:c + 1])

            # ---------------- finish ----------------
            minf = spool.tile([128, NT], F32, tag="minf")
            nc.vector.tensor_tensor(minf[:], minf2[:, 0:NT], minf2[:, NT:2 * NT],
                                    op=MIN)
            wm = spool.tile([128, NT], F32, tag="wm")
            nc.vector.scalar_tensor_tensor(wm[:], minf[:], 0.0, wN[:],
                                           op0=mybir.AluOpType.max, op1=MULT)
            nc.vector.tensor_scalar_max(minb[:], minb[:], 0.0)
            nc.vector.tensor_reduce(fin[:, 0:1], wm[:], axis=AX, op=ADD)
            nc.vector.tensor_reduce(fin[:, 1:2], wN[:], axis=AX, op=ADD)
            nc.vector.tensor_reduce(fin[:, 2:3], minb[:], axis=AX, op=ADD)

            with tc.tile_pool(name="psum_f", bufs=1, space="PSUM") as fps:
                ps3 = fps.tile([1, 3], F32, tag="ps3")
                nc.tensor.matmul(ps3[:], onescol[:], fin[:], start=True,
                                 stop=True)
                s3 = spool.tile([1, 3], F32, tag="s3")
                nc.vector.tensor_copy(s3[:], ps3[:])

            wsum = spool.tile([1, 1], F32, tag="wsum")
            nc.vector.tensor_scalar_max(wsum[:], s3[0:1, 1:2], CHAMFER_EPS)
            rwsum = spool.tile([1, 1], F32, tag="rwsum")
            nc.vector.reciprocal(rwsum[:], wsum[:])
            fwd = spool.tile([1, 1], F32, tag="fwd")
            nc.vector.tensor_tensor(fwd[:], s3[0:1, 0:1], rwsum[:], op=MULT)
            loss = spool.tile([1, 1], F32, tag="loss")
            nc.vector.scalar_tensor_tensor(loss[:], s3[0:1, 2:3], 1.0 / M,
                                           fwd[:], op0=MULT, op1=ADD)
            nc.sync.dma_start(out[:], loss[:])

    nc.compile()
    return nc


_NC_CACHE = {}


def get_nc():
    if "nc" not in _NC_CACHE:
        _NC_CACHE["nc"] = build_nc()
    return _NC_CACHE["nc"]


def make_in_maps(points, decoded_points, decoded_weights):
    in_maps = []
    for b in range(B):
        xT = np.ascontiguousarray(decoded_points[b].T).astype(np.float32)
        yT = np.ascontiguousarray(points[b].T).astype(np.float32)
        wT = np.ascontiguousarray(
            decoded_weights[b].reshape(NT, 128).T).astype(np.float32)
        sc = np.array([1, 1, 1, -2, -2, -2], dtype=np.float32).reshape(6, 1)
        in_maps.append({"xT": xT, "yT": yT, "wT": wT, "sc": sc})
    return in_maps


def kernel(points, decoded_points, decoded_weights):
    nc = get_nc()
    in_maps = make_in_maps(points, decoded_points, decoded_weights)
    res = run_bass_kernel_spmd(nc, in_maps, core_ids=list(range(B)))
    per_core = np.array([res.results[b]["loss"][0, 0] for b in range(B)],
                        dtype=np.float32)
    return np.asarray(per_core.mean(), dtype=np.float32)
